# revision 8
# baseline (speedup 1.0000x reference)
"""Cosine-similarity self-attention (Cos_Attn) on 8 Trainium2 NeuronCores.

Reference math (x: [C=512, W=64, H=64] fp32, N = W*H = 4096):
    q = x.reshape(C, N).T                  # [N, C]
    energy = q @ q.T                       # [N, N]
    cos    = energy / (|q_i| |q_j|)
    out    = softmax(cos, axis=-1)[None]   # [1, N, N]

Sharding: the N query rows are split across 8 cores (512 rows each). Every
core receives the full x (the keys) plus its own query column slice
xq = x[:, rows]; it computes its [512, N] slice of cos and the row softmax
locally; the host concatenates the 8 slices.

Per-core device algorithm (pipelined by the Tile scheduler, streamed in
512-column blocks):
  1. column norms: squares (GPSIMD/ACT) -> ones-matmul column-sum over the
     partition axis (PE, fp32r) -> rn = exp(-0.5*ln(ns)) on ACT. Ln/Exp/
     Square all live in the one `natural_log_exp_and_others` table set, so
     there is a single ACT table load (Sqrt/Rsqrt would thrash sets).
  2. xn = x * rn into dedicated float32r tiles (DVE) - every producer of an
     fp32r matmul operand must itself round to f32r, so normalized data
     gets its own tiles whose only writer is the DVE multiply.
  3. energy tiles = xnq^T @ xn (PE, fp32r: full rate at moving-dim 512),
     K=4x128 accumulated in PSUM.
  4. softmax: exp straight out of PSUM on ACT with accum_out row-sums
     (max-subtraction skipped: cos is bounded in [-1, 1]); reciprocal
     (DVE); row scale (DVE/GPSIMD); stream out.
"""

import numpy as np

_NCORES = 8
_P = 128

# set by the test harness only; the grading path keeps these defaults
TRACE = False
TRACE_CORES = None
LAST_RESULT = None

_built = None  # (nc, C, N)


def _build(C, N, RPC, sq_act_every=3):
    """Build the single-NEFF Bass/Tile program.

    Inputs:  x [C, N] (all keys), xq [C, RPC] (this core's query columns).
    Output:  out [RPC, N] = softmax rows for this core's queries.
    """
    from contextlib import ExitStack

    import concourse.tile as tile
    from concourse import bacc, mybir

    f32 = mybir.dt.float32
    f32r = mybir.dt.float32r
    AF = mybir.ActivationFunctionType
    AX = mybir.AxisListType
    OP = mybir.AluOpType

    P = _P
    KO = C // P              # contraction subtiles
    CB = 512                 # column block: one PSUM bank, fp32 moving-dim max
    NB = N // CB
    MT = RPC // P            # query row tiles per core
    HALF = max(KO // 2, 1)
    NH = KO // HALF          # square half-chunks per block

    nc = bacc.Bacc("TRN2", target_bir_lowering=False, debug=False)
    x_d = nc.dram_tensor("x", [C, N], f32, kind="ExternalInput")
    xq_d = nc.dram_tensor("xq", [C, RPC], f32, kind="ExternalInput")
    out_d = nc.dram_tensor("out", [RPC, N], f32, kind="ExternalOutput")

    x_r = x_d.ap().rearrange("(ko p) n -> p ko n", p=P)
    xq_r = xq_d.ap().rearrange("(ko p) m -> p ko m", p=P)
    out_r = out_d.ap().rearrange("(mo p) n -> p mo n", p=P)

    with tile.TileContext(nc) as tc, ExitStack() as ctx:
        persist = ctx.enter_context(tc.tile_pool(name="persist", bufs=1))
        temps = ctx.enter_context(tc.tile_pool(name="temps", bufs=3))
        psum = ctx.enter_context(tc.tile_pool(name="psum", bufs=6, space="PSUM"))

        xn_sb = persist.tile([P, KO, N], f32r)    # normalized keys (f32r)
        xnq_sb = persist.tile([P, KO, RPC], f32r)  # normalized queries (f32r)
        rn = persist.tile([P, N], f32)            # 1/|q_n|, replicated on parts
        rnq = persist.tile([P, RPC], f32)
        e = persist.tile([P, MT, N], f32)         # exp(cos); scaled in place
        sums = persist.tile([P, MT, NB], f32)     # per-(m, nb) exp row sums
        rs = persist.tile([P, MT], f32)
        rr = persist.tile([P, MT], f32)
        # memset can't emit f32r ISA; square a memset-1.0 f32 tile instead
        ones = persist.tile([P, P], f32r)
        ones_f = persist.tile([P, P], f32)
        nc.vector.memset(ones_f[:], 1.0)
        nc.scalar.activation(ones[:], ones_f[:], AF.Square)

        sq_state = [0]

        def colnorm(src, width, rn_out):
            """rn_out [P, width] <- 1/sqrt(colsum(src^2)); src [P, KO, width].

            Column sums over the partition axis via a ones-matmul (result
            lands replicated across all 128 partitions, the layout the later
            free-dim broadcasts need); then rn = exp(-0.5 * ln(ns)) on ACT.
            """
            ns_ps = psum.tile([P, width], f32, tag="ps", name="ns_ps")
            for h in range(NH):
                xsq = temps.tile([P, HALF, width], f32r, tag="xsq", name="xsq", bufs=2)
                src_h = src[:, h * HALF:(h + 1) * HALF, :]
                # alternate square work between ACT and GPSIMD (Square is a
                # filler function present in the exp/ln table set)
                if sq_state[0] % sq_act_every == 0:
                    nc.scalar.activation(xsq[:], src_h, AF.Square)
                else:
                    nc.gpsimd.tensor_mul(xsq[:], src_h, src_h)
                sq_state[0] += 1
                for k in range(HALF):
                    ko = h * HALF + k
                    nc.tensor.matmul(
                        ns_ps[:],
                        lhsT=ones[:],
                        rhs=xsq[:, k, :],
                        start=(ko == 0),
                        stop=(ko == KO - 1),
                    )
            nc.scalar.activation(rn_out, ns_ps[:], AF.Ln)
            nc.scalar.activation(rn_out, rn_out, AF.Exp, scale=-0.5)

        def normalize(dst, src, rn_ap, width):
            """dst [P, KO, width] (f32r) <- src * rn (rn broadcast over ko)."""
            try:
                rn_b = rn_ap[:, None, :].to_broadcast([P, KO, width])
                nc.vector.tensor_mul(dst, src, rn_b)
            except Exception:
                for k in range(KO):
                    nc.vector.tensor_mul(dst[:, k, :], src[:, k, :], rn_ap)

        # ---- query-side prologue ----
        xqr = temps.tile([P, KO, RPC], f32, tag="xqr", name="xqr", bufs=1)
        nc.sync.dma_start(xqr[:], xq_r)
        colnorm(xqr[:], RPC, rnq[:])
        normalize(xnq_sb[:], xqr[:], rnq[:], RPC)

        # ---- streamed main loop over column blocks ----
        for nb in range(NB):
            cs = slice(nb * CB, (nb + 1) * CB)
            xr = temps.tile([P, KO, CB], f32, tag="xr", name="xr", bufs=3)
            nc.sync.dma_start(xr[:], x_r[:, :, cs])
            colnorm(xr[:], CB, rn[:, cs])
            normalize(xn_sb[:, :, cs], xr[:], rn[:, cs], CB)
            for m in range(MT):
                ms = slice(m * P, (m + 1) * P)
                pt = psum.tile([P, CB], f32, tag="ps", name="pt")
                for k in range(KO):
                    nc.tensor.matmul(
                        pt[:],
                        lhsT=xnq_sb[:, k, ms],
                        rhs=xn_sb[:, k, cs],
                        start=(k == 0),
                        stop=(k == KO - 1),
                    )
                nc.scalar.activation(
                    e[:, m, cs], pt[:], AF.Exp,
                    accum_out=sums[:, m, nb:nb + 1],
                )

        # ---- tail: row-normalize, stream out ----
        OUT_CHUNK = min(N, 2048)
        for m in range(MT):
            nc.vector.tensor_reduce(
                rs[:, m:m + 1], sums[:, m, :], axis=AX.X, op=OP.add
            )
            nc.vector.reciprocal(rr[:, m:m + 1], rs[:, m:m + 1])
            for ci, c0 in enumerate(range(0, N, OUT_CHUNK)):
                ocs = slice(c0, c0 + OUT_CHUNK)
                eng = nc.vector if (m + ci) % 2 == 0 else nc.gpsimd
                eng.tensor_scalar_mul(e[:, m, ocs], e[:, m, ocs], rr[:, m:m + 1])
                nc.sync.dma_start(out_r[:, m, ocs], e[:, m, ocs])

    nc.compile()
    return nc


def kernel(**inputs) -> np.ndarray:
    global _built, LAST_RESULT
    x = np.ascontiguousarray(np.asarray(inputs["x"], dtype=np.float32))
    C, W, H = x.shape
    N = W * H
    RPC = N // _NCORES
    x2 = x.reshape(C, N)

    if _built is None or _built[1:] != (C, N):
        _built = (_build(C, N, RPC), C, N)
    nc = _built[0]

    from concourse import bass_utils

    in_maps = [
        {"x": x2, "xq": np.ascontiguousarray(x2[:, i * RPC:(i + 1) * RPC])}
        for i in range(_NCORES)
    ]
    kwargs = {}
    if TRACE:
        kwargs["trace"] = True
        if TRACE_CORES is not None:
            kwargs["trace_cores"] = list(TRACE_CORES)
    res = bass_utils.run_bass_kernel_spmd(
        nc, in_maps, core_ids=list(range(_NCORES)), **kwargs
    )
    LAST_RESULT = res
    out = np.concatenate([res.results[i]["out"] for i in range(_NCORES)], axis=0)
    return out.reshape(1, N, N)


# revision 11
# speedup vs baseline: 2.1218x; 2.1218x over previous
"""Cosine-similarity self-attention (Cos_Attn) on 8 Trainium2 NeuronCores.

Reference math (x: [C=512, W=64, H=64] fp32, N = W*H = 4096):
    q = x.reshape(C, N).T                  # [N, C]
    energy = q @ q.T                       # [N, N]
    cos    = energy / (|q_i| |q_j|)
    out    = softmax(cos, axis=-1)[None]   # [1, N, N]

Sharding: the N query rows are split across 8 cores (512 rows each). Every
core receives the full x (the keys) plus its own query column slice
xq = x[:, rows]; it computes its [512, N] slice of cos and the row softmax
locally; the host concatenates the 8 slices.

Per-core device pipeline (streamed in 512-column blocks):
  -  input x arrives in per-block DMAs spread over the three DMA-capable
     issue engines (SP + ACT hardware-DGE queues, GpSimd software-DGE) -
     a single queue measured only ~70-105 GB/s and paced the whole kernel.
  -  norms: squares (GPSIMD/DVE) -> ones-matmul column-sum (PE, bf16) into
     a 4-bank PSUM strip; rn = exp(-0.5*ln(ns)) on ACT in two 2048-wide
     chunks (Ln/Exp table-set switches cost ~1.3us each, so few wide chunks
     beat per-block ones).
  -  xn = x * rn into bf16 tiles (DVE; ko-broadcast of rn, a pattern
     validated on HW) - bf16 operands give full-rate PE + fast weight load.
  -  energy tiles = xnq^T @ xn (PE, bf16), K=4x128 accumulated in PSUM;
     softmax exp straight out of PSUM on ACT with accum_out row sums
     (max-subtraction skipped: cos is bounded in [-1, 1]).
  -  row scale 1/rowsum: per-partition scale on ACT (architectural
     free-affine operand) for half the row tiles, DVE multiply by a
     materialized [P,512] scale row (middle-dim stride-0 broadcast) for the
     other half; innermost-stride-0 APs and pointer-scalar TENSOR_SCALAR
     are avoided (measured wrong / 10x slow on HW).
"""

import numpy as np

_NCORES = 8
_P = 128

# set by the test harness only; the grading path keeps these defaults
TRACE = False
TRACE_CORES = None
LAST_RESULT = None

_built = None  # (nc, C, N)


def _build(C, N, RPC):
    """Build the single-NEFF Bass/Tile program.

    Inputs:  x [C, N] (all keys), xq [C, RPC] (this core's query columns).
    Output:  out [RPC, N] = softmax rows for this core's queries.
    """
    from contextlib import ExitStack

    import concourse.tile as tile
    from concourse import bacc, mybir

    f32 = mybir.dt.float32
    bf16 = mybir.dt.bfloat16
    AF = mybir.ActivationFunctionType
    AX = mybir.AxisListType
    OP = mybir.AluOpType

    P = _P
    KO = C // P              # contraction subtiles
    CB = 512                 # column block: one PSUM bank per energy tile
    NB = N // CB
    MT = RPC // P            # query row tiles per core
    HALF = max(KO // 2, 1)
    NH = KO // HALF          # square half-chunks per block
    RNC = min(NB, 4)         # rn chunk = RNC blocks (2048 cols)
    NRN = NB // RNC

    nc = bacc.Bacc("TRN2", target_bir_lowering=False, debug=False)
    x_d = nc.dram_tensor("x", [C, N], f32, kind="ExternalInput")
    xq_d = nc.dram_tensor("xq", [C, RPC], f32, kind="ExternalInput")
    out_d = nc.dram_tensor("out", [RPC, N], f32, kind="ExternalOutput")

    x_r = x_d.ap().rearrange("(ko p) n -> p ko n", p=P)
    xq_r = xq_d.ap().rearrange("(ko p) m -> p ko m", p=P)
    out_r = out_d.ap().rearrange("(mo p) n -> p mo n", p=P)

    with tile.TileContext(nc) as tc, ExitStack() as ctx:
        persist = ctx.enter_context(tc.tile_pool(name="persist", bufs=1))
        temps = ctx.enter_context(tc.tile_pool(name="temps", bufs=3))
        psum = ctx.enter_context(tc.tile_pool(name="psum", bufs=4, space="PSUM"))

        xn_sb = persist.tile([P, KO, N], bf16)     # normalized keys
        xnq_sb = persist.tile([P, KO, RPC], bf16)  # normalized queries
        rn = persist.tile([P, N], f32)             # 1/|q_n|, replicated on parts
        rnq = persist.tile([P, RPC], f32)
        e = persist.tile([P, MT, N], f32)          # exp(cos); scaled in place
        sums = persist.tile([P, MT, NB], f32)      # per-(m, nb) exp row sums
        rs = persist.tile([P, MT], f32)
        rr = persist.tile([P, MT], f32)
        ones = persist.tile([P, P], bf16)
        ones_row = persist.tile([P, CB], f32)
        nc.vector.memset(ones[:], 1.0)
        nc.vector.memset(ones_row[:], 1.0)

        # round-robin DMA issue over the three DMA-capable engines so the
        # transfers spread across independent DGE queues
        dma_engines = [nc.sync, nc.scalar, nc.gpsimd]
        dma_state = [0]

        def dma(out_ap, in_ap):
            eng = dma_engines[dma_state[0] % len(dma_engines)]
            dma_state[0] += 1
            eng.dma_start(out_ap, in_ap)

        sq_state = [0]

        def squares_and_colsum(src, width, ns_out):
            """ns_out [P, width] (PSUM) <- colsum over partitions of src^2."""
            for h in range(NH):
                xsq = temps.tile([P, HALF, width], bf16, tag="xsq", name="xsq",
                                 bufs=3)
                src_h = src[:, h * HALF:(h + 1) * HALF, :]
                # squares on GPSIMD (2 of 3) and DVE (1 of 3); ACT is the
                # busiest engine so it gets none
                if sq_state[0] % 3 == 2:
                    nc.vector.tensor_mul(xsq[:], src_h, src_h)
                else:
                    nc.gpsimd.tensor_mul(xsq[:], src_h, src_h)
                sq_state[0] += 1
                for k in range(HALF):
                    ko = h * HALF + k
                    nc.tensor.matmul(
                        ns_out,
                        lhsT=ones[:],
                        rhs=xsq[:, k, :],
                        start=(ko == 0),
                        stop=(ko == KO - 1),
                    )

        def rsqrt_act(dst, src_ps):
            """dst <- exp(-0.5*ln(src)); Ln/Exp live in one ACT table set."""
            nc.scalar.activation(dst, src_ps, AF.Ln)
            nc.scalar.activation(dst, dst, AF.Exp, scale=-0.5)

        def normalize(dst, src, rn_ap, width):
            """dst [P, KO, width] (bf16) <- src * rn (rn ko-broadcast)."""
            rn_b = rn_ap[:, None, :].to_broadcast([P, KO, width])
            nc.vector.tensor_mul(dst, src, rn_b)

        # ---- query-side prologue ----
        xqr = temps.tile([P, KO, RPC], f32, tag="xqr", name="xqr", bufs=1)
        dma(xqr[:], xq_r)
        nsq = psum.tile([P, RPC], f32, tag="ps", name="nsq")
        squares_and_colsum(xqr[:], RPC, nsq[:])
        rsqrt_act(rnq[:], nsq[:])
        normalize(xnq_sb[:], xqr[:], rnq[:], RPC)

        # ---- streamed main loop; rn computed per RNC-block chunk ----
        for rc in range(NRN):
            ns_strip = psum.tile([P, RNC, CB], f32, tag="nsb", name="ns_strip",
                                 bufs=1)
            xr_tiles = {}
            for j in range(RNC):
                nb = rc * RNC + j
                cs = slice(nb * CB, (nb + 1) * CB)
                xr = temps.tile([P, KO, CB], f32, tag="xr", name="xr", bufs=4)
                dma(xr[:, 0:HALF, :], x_r[:, 0:HALF, cs])
                dma(xr[:, HALF:KO, :], x_r[:, HALF:KO, cs])
                squares_and_colsum(xr[:], CB, ns_strip[:, j, :])
                xr_tiles[j] = xr  # raw block lives until normalize below
            ccs = slice(rc * RNC * CB, (rc + 1) * RNC * CB)
            rsqrt_act(rn[:, ccs], ns_strip[:].rearrange("p a b -> p (a b)"))
            # normalize + energy for the chunk's blocks
            for j in range(RNC):
                nb = rc * RNC + j
                cs = slice(nb * CB, (nb + 1) * CB)
                normalize(xn_sb[:, :, cs], xr_tiles[j][:], rn[:, cs], CB)
                for m in range(MT):
                    ms = slice(m * P, (m + 1) * P)
                    pt = psum.tile([P, CB], f32, tag="ps", name="pt")
                    for k in range(KO):
                        nc.tensor.matmul(
                            pt[:],
                            lhsT=xnq_sb[:, k, ms],
                            rhs=xn_sb[:, k, cs],
                            start=(k == 0),
                            stop=(k == KO - 1),
                        )
                    nc.scalar.activation(
                        e[:, m, cs], pt[:], AF.Exp,
                        accum_out=sums[:, m, nb:nb + 1],
                    )

        # ---- tail: row-normalize, stream out ----
        OUT_CHUNK = min(N, 2048)
        for m in range(MT):
            nc.vector.tensor_reduce(
                rs[:, m:m + 1], sums[:, m, :], axis=AX.X, op=OP.add
            )
            nc.vector.reciprocal(rr[:, m:m + 1], rs[:, m:m + 1])
            rr_m = rr[:, m:m + 1]
            if m % 2 == 1:
                # materialized scale row for the DVE middle-dim broadcast
                rr_row = temps.tile([P, CB], f32, tag="rr_row", name="rr_row",
                                    bufs=2)
                nc.scalar.activation(rr_row[:], ones_row[:], AF.Copy,
                                     scale=rr_m)
            for ci, c0 in enumerate(range(0, N, OUT_CHUNK)):
                ocs = slice(c0, c0 + OUT_CHUNK)
                if m % 2 == 0:
                    nc.scalar.activation(e[:, m, ocs], e[:, m, ocs], AF.Copy,
                                         scale=rr_m)
                else:
                    ev = e[:, m, ocs].rearrange("p (a b) -> p a b", b=CB)
                    rr_b = rr_row[:, None, :].to_broadcast(
                        [P, OUT_CHUNK // CB, CB])
                    nc.vector.tensor_mul(ev, ev, rr_b)
                dma(out_r[:, m, ocs], e[:, m, ocs])

    nc.compile()
    return nc


def kernel(**inputs) -> np.ndarray:
    global _built, LAST_RESULT
    x = np.ascontiguousarray(np.asarray(inputs["x"], dtype=np.float32))
    C, W, H = x.shape
    N = W * H
    RPC = N // _NCORES
    x2 = x.reshape(C, N)

    if _built is None or _built[1:] != (C, N):
        _built = (_build(C, N, RPC), C, N)
    nc = _built[0]

    from concourse import bass_utils

    in_maps = [
        {"x": x2, "xq": np.ascontiguousarray(x2[:, i * RPC:(i + 1) * RPC])}
        for i in range(_NCORES)
    ]
    kwargs = {}
    if TRACE:
        kwargs["trace"] = True
        if TRACE_CORES is not None:
            kwargs["trace_cores"] = list(TRACE_CORES)
    res = bass_utils.run_bass_kernel_spmd(
        nc, in_maps, core_ids=list(range(_NCORES)), **kwargs
    )
    LAST_RESULT = res
    out = np.concatenate([res.results[i]["out"] for i in range(_NCORES)], axis=0)
    return out.reshape(1, N, N)
